# revision 1
# baseline (speedup 1.0000x reference)
"""Multi-head attention (b=2, n=2048, dim=1024, h=16, dh=64) on 8 TRN2 NeuronCores.

Sharding: 32 (batch, head) pairs -> 8 cores x (1 batch, 4 heads). No collectives.
Per core:
  inputs : xT  [128, 8*2048] bf16 (x[b].T packed partition-major to match the
                                   SBUF layout: element (p, kt, n) = x[b].T[kt*128+p, n])
           wq  [1024, 256]  bf16  (q-columns of w_qkv for this core's 4 heads, pre-scaled by 1/8)
           wk  [1024, 256]  bf16
           wv  [1024, 256]  bf16
  output : out [4*65, 2048] f32   (per local head: rows 0-63 = unnormalized (attn@v)^T,
                                   row 64 = softmax denominator per query)
Host divides by the denominator and transposes back to [b, n, h*dh].

Device pipeline per core:
  qT/kT = (w.T @ x.T) in [d, n] layout, head-pairs packed 2x64 on partitions (bf16)
  V     = (x @ wv)    in [n, d] layout with a ones column appended (bf16)
  per head pair, per 512-wide query chunk, per 128-wide key block:
    S^T[j,i] = kT.T @ qT   (two K=64 matmuls packed into PE row-groups 0-63 / 64-127)
    A^T      = exp(S^T)    (one ACT instr over both heads' PSUM banks, f32 -> bf16)
    O^T     += [V|1].T @ A^T  (PSUM-accumulated over key blocks; row 64 = rowsum)
"""

import numpy as np
import ml_dtypes

B, N, DIM = 2, 2048, 1024
HEADS, DH = 16, 64
P = 128
KT = DIM // P          # 8 k-tiles
NT = N // P            # 16 n/j blocks
NCH = N // 512         # 4 chunks of 512
HL = 4                 # local heads per core
OROWS = HL * (DH + 1)  # 260 output rows per core

_CACHE = {}
LAST_RESULTS = None
TRACE = False


def _build_nc():
    from contextlib import ExitStack

    import concourse.bass as bass
    import concourse.tile as tile
    from concourse import bacc, mybir

    bf16 = mybir.dt.bfloat16
    f32 = mybir.dt.float32

    nc = bacc.Bacc("TRN2", target_bir_lowering=False)

    xT_d = nc.dram_tensor("xT", [P, KT * N], bf16, kind="ExternalInput")
    wq_d = nc.dram_tensor("wq", [DIM, HL * DH], bf16, kind="ExternalInput")
    wk_d = nc.dram_tensor("wk", [DIM, HL * DH], bf16, kind="ExternalInput")
    wv_d = nc.dram_tensor("wv", [DIM, HL * DH], bf16, kind="ExternalInput")
    out_d = nc.dram_tensor("out", [OROWS, N], f32, kind="ExternalOutput")

    # out rows viewed as [row-within-head, head, n] for packed output DMAs
    out_r = out_d[:, :].rearrange("(hh r) n -> r hh n", r=DH + 1)
    xT_r = xT_d[:, :].rearrange("p (kt n) -> p kt n", kt=KT)
    wq_r = wq_d[:, :].rearrange("(kt p) c -> p kt c", p=P)
    wk_r = wk_d[:, :].rearrange("(kt p) c -> p kt c", p=P)
    wv_r = wv_d[:, :].rearrange("(kt p) c -> p kt c", p=P)

    with tile.TileContext(nc) as tc, ExitStack() as ctx:
        sing = ctx.enter_context(tc.tile_pool(name="sing", bufs=1))
        spool = ctx.enter_context(
            tc.tile_pool(name="s_ps", bufs=3, space=bass.MemorySpace.PSUM)
        )
        opool = ctx.enter_context(
            tc.tile_pool(name="o_ps", bufs=1, space=bass.MemorySpace.PSUM)
        )
        apool = ctx.enter_context(tc.tile_pool(name="a_sb", bufs=14))
        copool = ctx.enter_context(tc.tile_pool(name="o_sb", bufs=4))

        # persistent SBUF tensors
        xT = sing.tile([P, KT, N], bf16, tag="xT")
        wq = sing.tile([P, KT, HL * DH], bf16, tag="wq")
        wk = sing.tile([P, KT, HL * DH], bf16, tag="wk")
        wv = sing.tile([P, KT, HL * DH], bf16, tag="wv")
        # head-pair packed projections: partitions 0-63 head A dims, 64-127 head B
        qT = [sing.tile([P, N], bf16, tag=f"qT{i}", name=f"qT{i}") for i in range(2)]
        kT = [sing.tile([P, N], bf16, tag=f"kT{i}", name=f"kT{i}") for i in range(2)]
        # V in [j, d] layout per j-block per head, with ones column at d=64
        v = sing.tile([P, NT, HL, DH + 1], bf16, tag="v")

        # input DMAs
        nc.gpsimd.dma_start(out=wk[:], in_=wk_r[:])
        nc.gpsimd.dma_start(out=wq[:], in_=wq_r[:])
        nc.gpsimd.dma_start(out=wv[:], in_=wv_r[:])
        # split the x transfer across both HWDGE rings (SP + ACT); the
        # Scalar engine is idle this early so its trigger cost is free
        # flatten each chunk to a 2D [128, 4096] AP so the per-partition 8KB
        # contiguous run is explicit (3D APs were split into small packets)
        xT_f = xT[:].rearrange("p kt n -> p (kt n)")
        for c in range(4):
            eng = nc.sync if c % 2 == 0 else nc.scalar
            eng.dma_start(
                out=xT_f[:, c * 4096 : (c + 1) * 4096],
                in_=xT_d[:, c * 4096 : (c + 1) * 4096],
            )

        # ---- projections ----
        # k, q: out[c, n] = w[:, c].T @ xT.  hp0 upfront; hp1 woven into
        # attention-hp0's periods (PE fills slack while ACT runs exp).
        def proj_unit(wt, dst, hp, nch):
            """Emit the 8 K-accumulated matmuls + copy for one 512-col chunk,
            returned as two 4-matmul halves so weaving stays fine-grained."""
            state = {}

            def half(h):
                if h == 0:
                    state["ps"] = spool.tile([P, 512], f32, tag="sp", name="ps")
                ps = state["ps"]
                for kt in range(4 * h, 4 * h + 4):
                    nc.tensor.matmul(
                        ps[:],
                        wt[:, kt, hp * P : (hp + 1) * P],
                        xT[:, kt, nch * 512 : (nch + 1) * 512],
                        start=(kt == 0),
                        stop=(kt == KT - 1),
                    )
                if h == 1:
                    nc.vector.tensor_copy(dst[:, nch * 512 : (nch + 1) * 512], ps[:])

            return [lambda: half(0), lambda: half(1)]

        # ones column of V (softmax denominator comes out of the PV matmul)
        nc.vector.memset(v[:, :, :, DH : DH + 1], 1.0)
        for unit in [proj_unit(wk, kT[0], 0, 0), proj_unit(wq, qT[0], 0, 0)]:
            for work in unit:
                work()

        # remaining projections are woven into the attention periods; each
        # woven chunk lands (in emission order) before the first scores
        # matmul that reads it.
        def full_unit(halves):
            return lambda: [h() for h in halves]

        woven = [full_unit(proj_unit(wq, qT[0], 0, 1))]
        woven_rest = []
        for nch in range(2, NCH):
            woven_rest.append(full_unit(proj_unit(wq, qT[0], 0, nch)))
        for wt, dst in ((wk, kT[1]), (wq, qT[1])):
            for nch in range(NCH):
                woven_rest.append(full_unit(proj_unit(wt, dst, 1, nch)))

        # V: out[n, c] = xT[:, ntile].T @ wv   -> [128 n, 256 c]
        def v_unit(nt):
            state = {}

            def half(h):
                if h == 0:
                    state["ps"] = spool.tile([P, HL * DH], f32, tag="sp", name="psv")
                ps = state["ps"]
                for kt in range(4 * h, 4 * h + 4):
                    nc.tensor.matmul(
                        ps[:],
                        xT[:, kt, nt * P : (nt + 1) * P],
                        wv[:, kt, :],
                        start=(kt == 0),
                        stop=(kt == KT - 1),
                    )
                if h == 1:
                    # scatter the 4 heads' 64 cols into the [NT, HL, 65] layout
                    nc.vector.tensor_copy(
                        v[:, nt, :, 0:DH],
                        ps[:].rearrange("p (h d) -> p h d", h=HL),
                    )

            return [lambda: half(0), lambda: half(1)]

        v_units = [full_unit(v_unit(nt)) for nt in range(NT)]

        # ---- attention ----
        # 8 blocks of 16 periods (one per (hp, ic)).  ACT runs one
        # [128, 1024] exp per period back-to-back; PE emits scores two
        # periods ahead (spool rotation) plus woven projection work; PV runs
        # as dense 8-matmul bursts every 4 periods (no exp-latency exposure).
        # Block 0 weaves the V projection (PV bursts shifted late until V is
        # ready); blocks 1+ weave the remaining q/k projections.
        blocks = [(hp, ic) for hp in range(2) for ic in range(NCH)]
        ats = {}
        opairs = {}
        sp_ahead = {}

        def emit_scores(b, jb):
            hp, ic = blocks[b]
            i0, j0 = ic * 512, jb * P
            sp = spool.tile([P, 1024], f32, tag="sp", name="sp")
            nc.tensor.matmul(
                sp[:, 0:512],
                kT[hp][0:DH, j0 : j0 + P],
                qT[hp][0:DH, i0 : i0 + 512],
                start=True, stop=True, tile_position=(0, 0),
            )
            nc.tensor.matmul(
                sp[:, 512:1024],
                kT[hp][DH:P, j0 : j0 + P],
                qT[hp][DH:P, i0 : i0 + 512],
                start=True, stop=True, tile_position=(64, 0),
            )
            return sp

        def emit_exp(b, jb, sp):
            at = apool.tile([P, 1024], bf16, tag="at", name="at")
            nc.scalar.activation(at[:], sp[:], mybir.ActivationFunctionType.Exp)
            ats[(b, jb)] = at

        def fetch_scores(b, jb):
            key = (b, jb)
            if key in sp_ahead:
                return sp_ahead.pop(key)
            return emit_scores(b, jb)

        def emit_pv_quarter(b, q):
            """PV matmuls for periods 4q..4q+3 of block b (dense burst)."""
            hp, ic = blocks[b]
            if q == 0:
                opairs[b] = (
                    opool.tile([DH + 1, 512], f32, tag="oA", name="oA"),
                    opool.tile([DH + 1, 512], f32, tag="oB", name="oB"),
                )
            oA, oB = opairs[b]
            for col, o in ((0, oA), (1, oB)):
                for jb in range(4 * q, 4 * q + 4):
                    nc.tensor.matmul(
                        o[:],
                        v[:, jb, 2 * hp + col, :],
                        ats[(b, jb)][:, 512 * col : 512 * col + 512],
                        start=(jb == 0), stop=(jb == NT - 1),
                    )
            for jb in range(4 * q, 4 * q + 4):
                del ats[(b, jb)]
            if q == 3:
                i0 = ic * 512
                os = copool.tile([DH + 1, 2, 512], f32, tag="os", name="os")
                nc.vector.tensor_copy(os[:, 0, :], oA[:])
                nc.vector.tensor_copy(os[:, 1, :], oB[:])
                nc.sync.dma_start(
                    out=out_r[:, 2 * hp : 2 * hp + 2, i0 : i0 + 512],
                    in_=os[:],
                )

        LA = 2  # scores lookahead depth
        nblocks = len(blocks)
        # prime the pipeline, then finish the kT01 projection chunks so the
        # first exp only waits on k01n0 + q01n0
        for nch in range(1, NCH):
            for work in proj_unit(wk, kT[0], 0, nch):
                work()
        for j in range(LA):
            sp_ahead[(0, j)] = emit_scores(0, j)
        for b in range(nblocks):
            for jb in range(NT):
                emit_exp(b, jb, fetch_scores(b, jb))
                la = jb + LA
                if la < NT:
                    if (b, la) not in sp_ahead:
                        sp_ahead[(b, la)] = emit_scores(b, la)
                elif b + 1 < nblocks:
                    sp_ahead[(b + 1, la - NT)] = emit_scores(b + 1, la - NT)
                if jb == NT - 1 and b + 1 < nblocks:
                    # boundary prefetch into the idle third spool slot: gives
                    # ACT a 3rd exp of cover across the 16-matmul PV burst
                    sp_ahead[(b + 1, LA)] = emit_scores(b + 1, LA)
                # woven PE filler
                p = b * NT + jb
                if b == 0:
                    if woven:
                        woven.pop(0)()
                    for _ in range(2):
                        if not woven and v_units:
                            v_units.pop(0)()
                elif woven_rest and (p - NT) % 5 == 4:
                    woven_rest.pop(0)()
                # PV bursts (block 0's deferred until the woven V is ready)
                if b == 0:
                    if jb in (8, 12):
                        emit_pv_quarter(0, (jb - 8) // 4)
                    elif jb == NT - 1:
                        while v_units:
                            v_units.pop(0)()
                        emit_pv_quarter(0, 2)
                        emit_pv_quarter(0, 3)
                elif jb % 4 == 0 and jb > 0:
                    emit_pv_quarter(b, jb // 4 - 1)
                elif jb == NT - 1:
                    emit_pv_quarter(b, 3)

    nc.compile()
    return nc


def _get_nc():
    if "nc" not in _CACHE:
        _CACHE["nc"] = _build_nc()
    return _CACHE["nc"]


def _prepare_in_maps(x, w_qkv):
    bf = ml_dtypes.bfloat16
    x = np.asarray(x, dtype=np.float32)
    w = np.asarray(w_qkv, dtype=np.float32)
    scale = DH ** -0.5
    in_maps = []
    xT_b = [
        np.ascontiguousarray(
            x[b].T.reshape(KT, P, N).transpose(1, 0, 2).reshape(P, KT * N)
        ).astype(bf)
        for b in range(B)
    ]
    for c in range(8):
        b, hg = divmod(c, 4)
        cs = slice(hg * HL * DH, (hg + 1) * HL * DH)
        in_maps.append(
            {
                "xT": xT_b[b],
                "wq": np.ascontiguousarray(w[:, cs] * scale).astype(bf),
                "wk": np.ascontiguousarray(w[:, 1024:2048][:, cs]).astype(bf),
                "wv": np.ascontiguousarray(w[:, 2048:3072][:, cs]).astype(bf),
            }
        )
    return in_maps


def _assemble(outs):
    full = np.empty((B, N, HEADS * DH), dtype=np.float32)
    for c in range(8):
        b, hg = divmod(c, 4)
        o = outs[c].reshape(HL, DH + 1, N)
        norm = o[:, :DH, :] / o[:, DH : DH + 1, :]  # [hl, d, n]
        full[b, :, hg * HL * DH : (hg + 1) * HL * DH] = norm.transpose(2, 0, 1).reshape(
            N, HL * DH
        )
    return full


def kernel(x, w_qkv):
    global LAST_RESULTS
    from concourse.bass_utils import run_bass_kernel_spmd

    nc = _get_nc()
    in_maps = _prepare_in_maps(x, w_qkv)
    last_err = None
    for _ in range(3):  # the runtime occasionally throws a transient device error
        try:
            res = run_bass_kernel_spmd(
                nc,
                in_maps,
                core_ids=list(range(8)),
                trace=TRACE,
                trace_cores=[0] if TRACE else None,
            )
            break
        except Exception as e:
            last_err = e
    else:
        raise last_err
    LAST_RESULTS = res
    return _assemble([r["out"] for r in res.results])



# revision 12
# speedup vs baseline: 1.2547x; 1.2547x over previous
"""Multi-head attention (b=2, n=2048, dim=1024, h=16, dh=64) on 8 TRN2 NeuronCores.

Sharding: 32 (batch, head) pairs -> 8 cores x (1 batch, 4 heads). No collectives.
Per core:
  inputs : xT  [128, 8*2048] bf16 (x[b].T packed n-chunk-major: element
                                   (p, nch, kt, n) = x[b].T[kt*128+p, nch*512+n])
           wq  [1024, 256]  bf16  (q-columns of w_qkv for this core's 4 heads, pre-scaled by 1/8)
           wk  [1024, 256]  bf16
           wv  [1024, 256]  bf16
  output : out [4*65, 2048] f32   (per local head: rows 0-63 = unnormalized (attn@v)^T,
                                   row 64 = softmax denominator per query)
Host divides by the denominator and transposes back to [b, n, h*dh].

Device pipeline per core:
  qT/kT = (w.T @ x.T) in [d, n] layout, head-pairs packed 2x64 on partitions (bf16)
  V     = (x @ wv)    in [n, d] layout with a ones column appended (bf16)
  per head pair, per 512-wide query chunk, per 128-wide key block:
    S^T[j,i] = kT.T @ qT   (two K=64 matmuls packed into PE row-groups 0-63 / 64-127)
    A^T      = exp(S^T)    (ACT f32->bf16 for most key blocks; for jb in S_DVE the
                            DVE computes a Schraudolph bit-hack exp instead:
                            bf16_bits = round(s*128*log2e + 16256 - C) as int16,
                            consumed by the PV matmul via a bf16 bitcast view)
    O^T     += [V|1].T @ A^T  (PSUM-accumulated over key blocks; row 64 = rowsum)

The xT DMA is n-chunk-major so the first projection (and hence the exp stream on
the critical ACT engine) starts after 1/4 of the x transfer instead of all of it.
"""

import numpy as np
import ml_dtypes

B, N, DIM = 2, 2048, 1024
HEADS, DH = 16, 64
P = 128
KT = DIM // P          # 8 k-tiles
NT = N // P            # 16 n/j blocks
NCH = N // 512         # 4 chunks of 512
HL = 4                 # local heads per core
OROWS = HL * (DH + 1)  # 260 output rows per core

# Schraudolph fast-exp constants (bf16 bit hack on the DVE engine):
#   bits = s * 128/ln(2) + (16256 - C [+0.5 for truncating converts])
# C ~= 7.33 zeroes the mean log-error so DVE-offloaded key blocks are
# unbiased relative to ACT-computed ones (the residual is a ~2% sawtooth
# that partially cancels in the softmax normalization).
SCH_A = 184.6650308540
SCH_C = 7.33
SCH_B = 16256.0 - SCH_C + 0.5
# Key blocks whose exp runs on the DVE, per attention block.  Blocks 0-1 are
# PE-bound (projection weave), so all their exps stay on ACT, which also keeps
# the projection-copy weave (a DVE op) free of same-queue deadlocks there.
S_DVE_EARLY = ()
S_DVE_B1 = (4, 10, 15)
S_DVE_MAIN = (2, 4, 7, 10, 12, 15)


def _s_dve(b):
    return S_DVE_EARLY if b == 0 else (S_DVE_B1 if b == 1 else S_DVE_MAIN)

_CACHE = {}
LAST_RESULTS = None
TRACE = False


def _build_nc():
    from contextlib import ExitStack

    import concourse.bass as bass
    import concourse.tile as tile
    from concourse import bacc, mybir

    bf16 = mybir.dt.bfloat16
    i16 = mybir.dt.int16
    f32 = mybir.dt.float32

    nc = bacc.Bacc("TRN2", target_bir_lowering=False)

    xT_d = nc.dram_tensor("xT", [P, KT * N], bf16, kind="ExternalInput")
    wq_d = nc.dram_tensor("wq", [DIM, HL * DH], bf16, kind="ExternalInput")
    wk_d = nc.dram_tensor("wk", [DIM, HL * DH], bf16, kind="ExternalInput")
    wv_d = nc.dram_tensor("wv", [DIM, HL * DH], bf16, kind="ExternalInput")
    out_d = nc.dram_tensor("out", [OROWS, N], f32, kind="ExternalOutput")

    # out rows viewed as [row-within-head, head, n] for packed output DMAs
    out_r = out_d[:, :].rearrange("(hh r) n -> r hh n", r=DH + 1)
    wq_r = wq_d[:, :].rearrange("(kt p) c -> p kt c", p=P)
    wk_r = wk_d[:, :].rearrange("(kt p) c -> p kt c", p=P)
    wv_r = wv_d[:, :].rearrange("(kt p) c -> p kt c", p=P)

    with tile.TileContext(nc) as tc, ExitStack() as ctx:
        sing = ctx.enter_context(tc.tile_pool(name="sing", bufs=1))
        spool = ctx.enter_context(
            tc.tile_pool(name="s_ps", bufs=3, space=bass.MemorySpace.PSUM)
        )
        opool = ctx.enter_context(
            tc.tile_pool(name="o_ps", bufs=1, space=bass.MemorySpace.PSUM)
        )
        apool = ctx.enter_context(tc.tile_pool(name="a_sb", bufs=14))
        copool = ctx.enter_context(tc.tile_pool(name="o_sb", bufs=4))

        # persistent SBUF tensors; xT is n-chunk-major: [p, nch, kt, n]
        xT = sing.tile([P, NCH, KT, 512], bf16, tag="xT")
        wq = sing.tile([P, KT, HL * DH], bf16, tag="wq")
        wk = sing.tile([P, KT, HL * DH], bf16, tag="wk")
        wv = sing.tile([P, KT, HL * DH], bf16, tag="wv")
        # head-pair packed projections: partitions 0-63 head A dims, 64-127 head B
        qT = [sing.tile([P, N], bf16, tag=f"qT{i}", name=f"qT{i}") for i in range(2)]
        kT = [sing.tile([P, N], bf16, tag=f"kT{i}", name=f"kT{i}") for i in range(2)]
        # V in [j, d] layout per j-block per head, with ones column at d=64
        v = sing.tile([P, NT, HL, DH + 1], bf16, tag="v")

        # input DMAs: weights via the software DGE; x in four n-chunk pieces
        # alternating across the two HWDGE rings so chunk 0 lands first and
        # the first projection can start after ~1/4 of the transfer.
        nc.gpsimd.dma_start(out=wk[:], in_=wk_r[:])
        nc.gpsimd.dma_start(out=wq[:], in_=wq_r[:])
        nc.gpsimd.dma_start(out=wv[:], in_=wv_r[:])
        xT_f = xT[:].rearrange("p c kt n -> p (c kt n)")
        for c, eng in ((0, nc.sync), (1, nc.scalar), (2, nc.sync), (3, nc.scalar)):
            eng.dma_start(
                out=xT_f[:, c * 4096 : (c + 1) * 4096],
                in_=xT_d[:, c * 4096 : (c + 1) * 4096],
            )

        # ---- spool slot-consumer tracking ----
        # Every spool.tile() allocation is logged with who consumes the tile.
        # A projection/V copy runs on the DVE; its matmuls wait for the slot
        # (3 allocations back) to be freed by that slot's consumer.  If that
        # consumer were a DVE Schraudolph emitted LATER than the copy, the DVE
        # queue would deadlock on itself, so the weave only emits a unit when
        # the slot's pending consumer is an ACT exp or an already-emitted DVE
        # op.  Consumers are tagged (kind, period).
        sp_log = []

        def sp_alloc(shape, consumer):
            sp_log.append(consumer)
            return spool.tile(shape, f32, tag="sp", name="sp")

        def weave_safe(cur_period):
            if len(sp_log) < 3:
                return True
            kind, period = sp_log[-3]
            return kind != "schr" or period <= cur_period

        # ---- projections ----
        done = set()  # emitted projection/V units, for deadline asserts

        # k, q: out[c, n] = w[:, c].T @ xT, one 512-col chunk at a time.
        def proj_unit(wt, dst, hp, nch, key):
            """Emit the 8 K-accumulated matmuls + copy for one 512-col chunk."""

            def work(cur_period):
                ps = sp_alloc([P, 512], ("copy", cur_period))
                for kt in range(KT):
                    nc.tensor.matmul(
                        ps[:],
                        wt[:, kt, hp * P : (hp + 1) * P],
                        xT[:, nch, kt, :],
                        start=(kt == 0),
                        stop=(kt == KT - 1),
                    )
                nc.vector.tensor_copy(dst[:, nch * 512 : (nch + 1) * 512], ps[:])
                done.add(key)

            return work

        # V: out[n, c] = xT[:, ntile].T @ wv   -> [128 n, 256 c]
        def v_unit(nt):
            def work(cur_period):
                ps = sp_alloc([P, HL * DH], ("copy", cur_period))
                nch, sub = divmod(nt, NCH)
                for kt in range(KT):
                    nc.tensor.matmul(
                        ps[:],
                        xT[:, nch, kt, sub * P : (sub + 1) * P],
                        wv[:, kt, :],
                        start=(kt == 0),
                        stop=(kt == KT - 1),
                    )
                # scatter the 4 heads' 64 cols into the [NT, HL, 65] layout
                nc.vector.tensor_copy(
                    v[:, nt, :, 0:DH],
                    ps[:].rearrange("p (h d) -> p h d", h=HL),
                )
                done.add(("v", nt))

            return work

        # ones column of V (softmax denominator comes out of the PV matmul)
        nc.vector.memset(v[:, :, :, DH : DH + 1], 1.0)

        # head: only what the first exp needs, then prime the scores pipeline
        proj_unit(wk, kT[0], 0, 0, ("k", 0, 0))((-1, 0))
        proj_unit(wq, qT[0], 0, 0, ("q", 0, 0))((-1, 1))

        # ---- attention ----
        # 8 blocks of 16 periods (one per (hp, ic)).  The exp stream paces the
        # kernel: ACT runs [128, 1024] exps back-to-back while the DVE handles
        # the S_DVE key blocks concurrently via the bit-hack.  PE emits scores
        # two periods ahead (spool rotation), weaves the remaining projection
        # work, and runs PV as dense bursts with quarter q3 deferred into the
        # next block so it never waits on a just-finished exp.
        blocks = [(hp, ic) for hp in range(2) for ic in range(NCH)]
        ats = {}
        opairs = {}
        sp_ahead = {}

        def emit_scores(b, jb):
            hp, ic = blocks[b]
            i0, j0 = ic * 512, jb * P
            assert ("k", hp, (jb * P) // 512) in done, ("k-chunk", b, jb)
            assert ("q", hp, ic) in done, ("q-chunk", b, jb)
            kind = "schr" if jb in _s_dve(b) else "exp"
            sp = sp_alloc([P, 1024], (kind, (b, jb)))
            nc.tensor.matmul(
                sp[:, 0:512],
                kT[hp][0:DH, j0 : j0 + P],
                qT[hp][0:DH, i0 : i0 + 512],
                start=True, stop=True, tile_position=(0, 0),
            )
            nc.tensor.matmul(
                sp[:, 512:1024],
                kT[hp][DH:P, j0 : j0 + P],
                qT[hp][DH:P, i0 : i0 + 512],
                start=True, stop=True, tile_position=(64, 0),
            )
            return sp

        def emit_exp(b, jb, sp):
            if jb in _s_dve(b):
                at = apool.tile([P, 1024], i16, tag="at", name="ats")
                nc.vector.tensor_scalar(
                    out=at[:],
                    in0=sp[:],
                    scalar1=SCH_A,
                    scalar2=SCH_B,
                    op0=mybir.AluOpType.mult,
                    op1=mybir.AluOpType.add,
                )
                ats[(b, jb)] = (at, True)
            else:
                at = apool.tile([P, 1024], bf16, tag="at", name="at")
                nc.scalar.activation(at[:], sp[:], mybir.ActivationFunctionType.Exp)
                ats[(b, jb)] = (at, False)

        def fetch_scores(b, jb):
            key = (b, jb)
            if key in sp_ahead:
                return sp_ahead.pop(key)
            return emit_scores(b, jb)

        def at_rhs(b, jb, col):
            t, is_i16 = ats[(b, jb)]
            rhs = t[:, 512 * col : 512 * col + 512]
            return rhs.bitcast(bf16) if is_i16 else rhs

        def emit_pv(b, jbs, last=False):
            """PV matmuls of block b for the given key blocks (dense burst)."""
            hp, ic = blocks[b]
            if b not in opairs:
                opairs[b] = opool.tile([DH + 1, 2, 512], f32, tag="oT", name="oT")
            oT = opairs[b]
            for jb in jbs:
                assert ("v", jb) in done, ("v-unit", b, jb)
            for col in range(2):
                for jb in jbs:
                    nc.tensor.matmul(
                        oT[:, col, :],
                        v[:, jb, 2 * hp + col, :],
                        at_rhs(b, jb, col),
                        start=(jb == 0), stop=(jb == NT - 1),
                    )
            for jb in jbs:
                del ats[(b, jb)]
            if last:
                i0 = ic * 512
                os = copool.tile([DH + 1, 2, 512], f32, tag="os", name="os")
                nc.vector.tensor_copy(os[:], oT[:])
                nc.sync.dma_start(
                    out=out_r[:, 2 * hp : 2 * hp + 2, i0 : i0 + 512],
                    in_=os[:],
                )

        # woven PE filler, scheduled by (block, period) but drained through a
        # pending queue gated by weave_safe(): projection chunks and V units
        # land (in emission order) before the first matmul that reads them.
        # q0c1 is needed by scores(b1) emitted at (b0, 14); v0-3 by PV(b0, q0)
        # at (b0, 8); v8-15 by PV(b0, q2/q3) at (b1, 0/2); kT[1] and qT[1] by
        # the hp=1 blocks starting at period 62.
        weave = {}
        b0_units = [
            proj_unit(wk, kT[0], 0, 1, ("k", 0, 1)),
            proj_unit(wk, kT[0], 0, 2, ("k", 0, 2)),
            proj_unit(wk, kT[0], 0, 3, ("k", 0, 3)),
            proj_unit(wq, qT[0], 0, 1, ("q", 0, 1)),
        ] + [v_unit(nt) for nt in range(12)]
        for i, u in enumerate(b0_units):
            weave.setdefault((0, i), []).append(u)
        # v12-15 must land before PV(b0, q3) at period (1, 2)
        b1_places = [(1, 0), (1, 0), (1, 1), (1, 1), (1, 3)]
        b1_units = [
            v_unit(12), v_unit(13), v_unit(14), v_unit(15),
            proj_unit(wq, qT[0], 0, 2, ("q", 0, 2)),
        ]
        for place, u in zip(b1_places, b1_units):
            weave.setdefault(place, []).append(u)
        rest = [proj_unit(wq, qT[0], 0, 3, ("q", 0, 3))]
        for wt, dst, nm in ((wk, kT[1], "k"), (wq, qT[1], "q")):
            for nch in range(NCH):
                rest.append(proj_unit(wt, dst, 1, nch, (nm, 1, nch)))
        for i, u in enumerate(rest):
            p = 32 + 5 * i  # every 5th period from block 2 onward
            weave.setdefault((p // NT, p % NT), []).append(u)

        LA = 2  # scores lookahead depth
        nblocks = len(blocks)
        for j in range(LA):
            sp_ahead[(0, j)] = emit_scores(0, j)
        pending = []
        for b in range(nblocks):
            for jb in range(NT):
                emit_exp(b, jb, fetch_scores(b, jb))
                la = jb + LA
                if la < NT:
                    if (b, la) not in sp_ahead:
                        sp_ahead[(b, la)] = emit_scores(b, la)
                elif b + 1 < nblocks:
                    sp_ahead[(b + 1, la - NT)] = emit_scores(b + 1, la - NT)
                if jb == NT - 1 and b + 1 < nblocks:
                    # boundary prefetch into the idle third spool slot: keeps
                    # the exp stream covered across the PV bursts
                    sp_ahead[(b + 1, LA)] = emit_scores(b + 1, LA)
                # woven PE filler (deadlock-gated; up to 2 units per period)
                pending.extend(weave.pop((b, jb), ()))
                emitted = 0
                while pending and emitted < 2 and weave_safe((b, jb)):
                    pending.pop(0)((b, jb))
                    emitted += 1
                # PV bursts: q0-q2 inside the block, q3 deferred into the next
                # block (reads 4-period-old tiles, so no exp-latency exposure);
                # the final block's q3 is split so the tail is 2 matmuls long.
                if jb == 0 and b > 0:
                    emit_pv(b - 1, range(8, 12))
                elif jb == 2 and b > 0:
                    emit_pv(b - 1, range(12, NT), last=True)
                if jb in (8, 12):
                    emit_pv(b, range(jb - 8, jb - 4))
                if b == nblocks - 1:
                    if jb == 13:
                        emit_pv(b, range(8, 12))
                    elif jb == 14:
                        emit_pv(b, (12, 13))
                    elif jb == NT - 1:
                        emit_pv(b, (14, 15), last=True)
        assert not pending and not weave, (len(pending), sorted(weave))

    nc.compile()
    return nc


def _get_nc():
    if "nc" not in _CACHE:
        _CACHE["nc"] = _build_nc()
    return _CACHE["nc"]


def _prepare_in_maps(x, w_qkv):
    bf = ml_dtypes.bfloat16
    x = np.asarray(x, dtype=np.float32)
    w = np.asarray(w_qkv, dtype=np.float32)
    scale = DH ** -0.5
    in_maps = []
    # xT n-chunk-major: element (p, nch, kt, n) = x[b].T[kt*128+p, nch*512+n]
    xT_b = [
        np.ascontiguousarray(
            x[b].T.reshape(KT, P, NCH, 512).transpose(1, 2, 0, 3).reshape(P, KT * N)
        ).astype(bf)
        for b in range(B)
    ]
    for c in range(8):
        b, hg = divmod(c, 4)
        cs = slice(hg * HL * DH, (hg + 1) * HL * DH)
        in_maps.append(
            {
                "xT": xT_b[b],
                "wq": np.ascontiguousarray(w[:, cs] * scale).astype(bf),
                "wk": np.ascontiguousarray(w[:, 1024:2048][:, cs]).astype(bf),
                "wv": np.ascontiguousarray(w[:, 2048:3072][:, cs]).astype(bf),
            }
        )
    return in_maps


def _assemble(outs):
    full = np.empty((B, N, HEADS * DH), dtype=np.float32)
    for c in range(8):
        b, hg = divmod(c, 4)
        o = outs[c].reshape(HL, DH + 1, N)
        norm = o[:, :DH, :] / o[:, DH : DH + 1, :]  # [hl, d, n]
        full[b, :, hg * HL * DH : (hg + 1) * HL * DH] = norm.transpose(2, 0, 1).reshape(
            N, HL * DH
        )
    return full


def kernel(x, w_qkv):
    global LAST_RESULTS
    from concourse.bass_utils import run_bass_kernel_spmd

    nc = _get_nc()
    in_maps = _prepare_in_maps(x, w_qkv)
    last_err = None
    for _ in range(3):  # the runtime occasionally throws a transient device error
        try:
            res = run_bass_kernel_spmd(
                nc,
                in_maps,
                core_ids=list(range(8)),
                trace=TRACE,
                trace_cores=[0] if TRACE else None,
            )
            break
        except Exception as e:
            last_err = e
    else:
        raise last_err
    LAST_RESULTS = res
    return _assemble([r["out"] for r in res.results])


# revision 14
# speedup vs baseline: 1.2903x; 1.0283x over previous
"""Multi-head attention (b=2, n=2048, dim=1024, h=16, dh=64) on 8 TRN2 NeuronCores.

Sharding: 32 (batch, head) pairs -> 8 cores x (1 batch, 4 heads). No collectives.
Per core:
  inputs : xT  [128, 8*2048] bf16 (x[b].T packed n-chunk-major: element
                                   (p, nch, kt, n) = x[b].T[kt*128+p, nch*512+n])
           wq  [1024, 256]  bf16  (q-columns of w_qkv for this core's 4 heads, pre-scaled by 1/8)
           wk  [1024, 256]  bf16
           wv  [1024, 256]  bf16
  output : out [4*65, 2048] f32   (per local head: rows 0-63 = unnormalized (attn@v)^T,
                                   row 64 = softmax denominator per query)
Host divides by the denominator and transposes back to [b, n, h*dh].

Device pipeline per core:
  qT/kT = (w.T @ x.T) in [d, n] layout, head-pairs packed 2x64 on partitions (bf16)
  V     = (x @ wv)    in [n, d] layout with a ones column appended (bf16)
  per head pair, per 512-wide query chunk, per 128-wide key block:
    S^T[j,i] = kT.T @ qT   (two K=64 matmuls packed into PE row-groups 0-63 / 64-127)
    A^T      = exp(S^T)    (ACT f32->bf16 for most key blocks; for jb in S_DVE the
                            DVE computes a Schraudolph bit-hack exp instead:
                            bf16_bits = round(s*128*log2e + 16256 - C) as int16,
                            consumed by the PV matmul via a bf16 bitcast view)
    O^T     += [V|1].T @ A^T  (PSUM-accumulated over key blocks; row 64 = rowsum)

The xT DMA is n-chunk-major so the first projection (and hence the exp stream on
the critical ACT engine) starts after 1/4 of the x transfer instead of all of it.
"""

import numpy as np
import ml_dtypes

B, N, DIM = 2, 2048, 1024
HEADS, DH = 16, 64
P = 128
KT = DIM // P          # 8 k-tiles
NT = N // P            # 16 n/j blocks
NCH = N // 512         # 4 chunks of 512
HL = 4                 # local heads per core
OROWS = HL * (DH + 1)  # 260 output rows per core

# Schraudolph fast-exp constants (bf16 bit hack on the DVE engine):
#   bits = s * 128/ln(2) + (16256 - C [+0.5 for truncating converts])
# C ~= 7.33 zeroes the mean log-error so DVE-offloaded key blocks are
# unbiased relative to ACT-computed ones (the residual is a ~2% sawtooth
# that partially cancels in the softmax normalization).
SCH_A = 184.6650308540
SCH_C = 7.33
SCH_B = 16256.0 - SCH_C + 0.5
# Key blocks whose exp runs on the DVE, per attention block.  Blocks 0-1 are
# PE-bound (projection weave), so all their exps stay on ACT, which also keeps
# the projection-copy weave (a DVE op) free of same-queue deadlocks there.
S_DVE_EARLY = (5, 11)
S_DVE_MAIN = (4, 9, 14)


def _s_dve(b):
    return S_DVE_EARLY if b <= 1 else S_DVE_MAIN

_CACHE = {}
LAST_RESULTS = None
TRACE = False


def _build_nc():
    from contextlib import ExitStack

    import concourse.bass as bass
    import concourse.tile as tile
    from concourse import bacc, mybir

    bf16 = mybir.dt.bfloat16
    i16 = mybir.dt.int16
    f32 = mybir.dt.float32

    nc = bacc.Bacc("TRN2", target_bir_lowering=False)

    xT_d = nc.dram_tensor("xT", [P, KT * N], bf16, kind="ExternalInput")
    wq_d = nc.dram_tensor("wq", [DIM, HL * DH], bf16, kind="ExternalInput")
    wk_d = nc.dram_tensor("wk", [DIM, HL * DH], bf16, kind="ExternalInput")
    wv_d = nc.dram_tensor("wv", [DIM, HL * DH], bf16, kind="ExternalInput")
    out_d = nc.dram_tensor("out", [OROWS, N], f32, kind="ExternalOutput")

    # out rows viewed as [row-within-head, head, n] for packed output DMAs
    out_r = out_d[:, :].rearrange("(hh r) n -> r hh n", r=DH + 1)
    wq_r = wq_d[:, :].rearrange("(kt p) c -> p kt c", p=P)
    wk_r = wk_d[:, :].rearrange("(kt p) c -> p kt c", p=P)
    wv_r = wv_d[:, :].rearrange("(kt p) c -> p kt c", p=P)

    with tile.TileContext(nc) as tc, ExitStack() as ctx:
        sing = ctx.enter_context(tc.tile_pool(name="sing", bufs=1))
        spool = ctx.enter_context(
            tc.tile_pool(name="s_ps", bufs=3, space=bass.MemorySpace.PSUM)
        )
        opool = ctx.enter_context(
            tc.tile_pool(name="o_ps", bufs=1, space=bass.MemorySpace.PSUM)
        )
        apool = ctx.enter_context(tc.tile_pool(name="a_sb", bufs=14))
        copool = ctx.enter_context(tc.tile_pool(name="o_sb", bufs=4))

        # persistent SBUF tensors; xT is n-chunk-major: [p, nch, kt, n]
        xT = sing.tile([P, NCH, KT, 512], bf16, tag="xT")
        wq = sing.tile([P, KT, HL * DH], bf16, tag="wq")
        wk = sing.tile([P, KT, HL * DH], bf16, tag="wk")
        wv = sing.tile([P, KT, HL * DH], bf16, tag="wv")
        # head-pair packed projections: partitions 0-63 head A dims, 64-127 head B
        qT = [sing.tile([P, N], bf16, tag=f"qT{i}", name=f"qT{i}") for i in range(2)]
        kT = [sing.tile([P, N], bf16, tag=f"kT{i}", name=f"kT{i}") for i in range(2)]
        # V in [j, d] layout per j-block per head, with ones column at d=64
        v = sing.tile([P, NT, HL, DH + 1], bf16, tag="v")

        # input DMAs: STRICTLY SERIAL on one HWDGE ring in dependency order
        # (wk, x-chunk0, wq, then the rest).  Splitting across rings makes the
        # transfers share HBM bandwidth round-robin, so chunk 0 — which gates
        # the first projection and hence the whole exp stream — would finish
        # last instead of first.  wv rides the software DGE; it isn't needed
        # until the V-projection weave.
        nc.gpsimd.dma_start(out=wv[:], in_=wv_r[:])
        xT_f = xT[:].rearrange("p c kt n -> p (c kt n)")
        nc.sync.dma_start(out=wk[:], in_=wk_r[:])
        nc.sync.dma_start(out=xT_f[:, 0:4096], in_=xT_d[:, 0:4096])
        nc.sync.dma_start(out=wq[:], in_=wq_r[:])
        for c in (1, 2, 3):
            nc.sync.dma_start(
                out=xT_f[:, c * 4096 : (c + 1) * 4096],
                in_=xT_d[:, c * 4096 : (c + 1) * 4096],
            )

        # ---- spool slot-consumer tracking ----
        # Every spool.tile() allocation is logged with who consumes the tile.
        # A projection/V copy runs on the DVE; its matmuls wait for the slot
        # (3 allocations back) to be freed by that slot's consumer.  If that
        # consumer were a DVE Schraudolph emitted LATER than the copy, the DVE
        # queue would deadlock on itself, so the weave only emits a unit when
        # the slot's pending consumer is an ACT exp or an already-emitted DVE
        # op.  Consumers are tagged (kind, period).
        sp_log = []

        def sp_alloc(shape, consumer):
            sp_log.append(consumer)
            return spool.tile(shape, f32, tag="sp", name="sp")

        def weave_safe(cur_period):
            if len(sp_log) < 3:
                return True
            kind, period = sp_log[-3]
            return kind != "schr" or period <= cur_period

        # ---- projections ----
        done = set()  # emitted projection/V units, for deadline asserts

        # k, q: out[c, n] = w[:, c].T @ xT, one 512-col chunk at a time.
        def proj_unit(wt, dst, hp, nch, key):
            """Emit the 8 K-accumulated matmuls + copy for one 512-col chunk."""

            def work(cur_period):
                ps = sp_alloc([P, 512], ("copy", cur_period))
                for kt in range(KT):
                    nc.tensor.matmul(
                        ps[:],
                        wt[:, kt, hp * P : (hp + 1) * P],
                        xT[:, nch, kt, :],
                        start=(kt == 0),
                        stop=(kt == KT - 1),
                    )
                nc.vector.tensor_copy(dst[:, nch * 512 : (nch + 1) * 512], ps[:])
                done.add(key)

            return work

        # V: out[n, c] = xT[:, ntile].T @ wv   -> [128 n, 256 c]
        def v_unit(nt):
            def work(cur_period):
                ps = sp_alloc([P, HL * DH], ("copy", cur_period))
                nch, sub = divmod(nt, NCH)
                for kt in range(KT):
                    nc.tensor.matmul(
                        ps[:],
                        xT[:, nch, kt, sub * P : (sub + 1) * P],
                        wv[:, kt, :],
                        start=(kt == 0),
                        stop=(kt == KT - 1),
                    )
                # scatter the 4 heads' 64 cols into the [NT, HL, 65] layout
                nc.vector.tensor_copy(
                    v[:, nt, :, 0:DH],
                    ps[:].rearrange("p (h d) -> p h d", h=HL),
                )
                done.add(("v", nt))

            return work

        # ones column of V (softmax denominator comes out of the PV matmul)
        nc.vector.memset(v[:, :, :, DH : DH + 1], 1.0)

        # head: only what the first exp needs, then prime the scores pipeline
        proj_unit(wk, kT[0], 0, 0, ("k", 0, 0))((-1, 0))
        proj_unit(wq, qT[0], 0, 0, ("q", 0, 0))((-1, 1))

        # ---- attention ----
        # 8 blocks of 16 periods (one per (hp, ic)).  The exp stream paces the
        # kernel: ACT runs [128, 1024] exps back-to-back while the DVE handles
        # the S_DVE key blocks concurrently via the bit-hack.  PE emits scores
        # two periods ahead (spool rotation), weaves the remaining projection
        # work, and runs PV as dense bursts with quarter q3 deferred into the
        # next block so it never waits on a just-finished exp.
        blocks = [(hp, ic) for hp in range(2) for ic in range(NCH)]
        ats = {}
        opairs = {}
        sp_ahead = {}

        def emit_scores(b, jb):
            hp, ic = blocks[b]
            i0, j0 = ic * 512, jb * P
            assert ("k", hp, (jb * P) // 512) in done, ("k-chunk", b, jb)
            assert ("q", hp, ic) in done, ("q-chunk", b, jb)
            kind = "schr" if jb in _s_dve(b) else "exp"
            sp = sp_alloc([P, 1024], (kind, (b, jb)))
            nc.tensor.matmul(
                sp[:, 0:512],
                kT[hp][0:DH, j0 : j0 + P],
                qT[hp][0:DH, i0 : i0 + 512],
                start=True, stop=True, tile_position=(0, 0),
            )
            nc.tensor.matmul(
                sp[:, 512:1024],
                kT[hp][DH:P, j0 : j0 + P],
                qT[hp][DH:P, i0 : i0 + 512],
                start=True, stop=True, tile_position=(64, 0),
            )
            return sp

        def emit_exp(b, jb, sp):
            if jb in _s_dve(b):
                at = apool.tile([P, 1024], i16, tag="at", name="ats")
                nc.vector.tensor_scalar(
                    out=at[:],
                    in0=sp[:],
                    scalar1=SCH_A,
                    scalar2=SCH_B,
                    op0=mybir.AluOpType.mult,
                    op1=mybir.AluOpType.add,
                )
                ats[(b, jb)] = (at, True)
            else:
                at = apool.tile([P, 1024], bf16, tag="at", name="at")
                nc.scalar.activation(at[:], sp[:], mybir.ActivationFunctionType.Exp)
                ats[(b, jb)] = (at, False)

        def fetch_scores(b, jb):
            key = (b, jb)
            if key in sp_ahead:
                return sp_ahead.pop(key)
            return emit_scores(b, jb)

        def at_rhs(b, jb, col):
            t, is_i16 = ats[(b, jb)]
            rhs = t[:, 512 * col : 512 * col + 512]
            return rhs.bitcast(bf16) if is_i16 else rhs

        def emit_pv(b, jbs, last=False):
            """PV matmuls of block b for the given key blocks (dense burst)."""
            hp, ic = blocks[b]
            if b not in opairs:
                opairs[b] = opool.tile([DH + 1, 2, 512], f32, tag="oT", name="oT")
            oT = opairs[b]
            for jb in jbs:
                assert ("v", jb) in done, ("v-unit", b, jb)
            for col in range(2):
                for jb in jbs:
                    nc.tensor.matmul(
                        oT[:, col, :],
                        v[:, jb, 2 * hp + col, :],
                        at_rhs(b, jb, col),
                        start=(jb == 0), stop=(jb == NT - 1),
                    )
            for jb in jbs:
                del ats[(b, jb)]
            if last:
                i0 = ic * 512
                os = copool.tile([DH + 1, 2, 512], f32, tag="os", name="os")
                nc.vector.tensor_copy(os[:], oT[:])
                nc.sync.dma_start(
                    out=out_r[:, 2 * hp : 2 * hp + 2, i0 : i0 + 512],
                    in_=os[:],
                )

        # woven PE filler, scheduled by (block, period) but drained through a
        # pending queue gated by weave_safe(): projection chunks and V units
        # land (in emission order) before the first matmul that reads them.
        # q0c1 is needed by scores(b1) emitted at (b0, 14); v0-3 by PV(b0, q0)
        # at (b0, 8); v8-15 by PV(b0, q2/q3) at (b1, 0/2); kT[1] and qT[1] by
        # the hp=1 blocks starting at period 62.
        weave = {}
        b0_units = [
            proj_unit(wk, kT[0], 0, 1, ("k", 0, 1)),
            proj_unit(wk, kT[0], 0, 2, ("k", 0, 2)),
            proj_unit(wk, kT[0], 0, 3, ("k", 0, 3)),
            proj_unit(wq, qT[0], 0, 1, ("q", 0, 1)),
        ] + [v_unit(nt) for nt in range(12)]
        for i, u in enumerate(b0_units):
            weave.setdefault((0, i), []).append(u)
        # v12-15 must land before PV(b0, q3) at period (1, 2)
        b1_places = [(1, 0), (1, 0), (1, 1), (1, 1), (1, 3)]
        b1_units = [
            v_unit(12), v_unit(13), v_unit(14), v_unit(15),
            proj_unit(wq, qT[0], 0, 2, ("q", 0, 2)),
        ]
        for place, u in zip(b1_places, b1_units):
            weave.setdefault(place, []).append(u)
        rest = [proj_unit(wq, qT[0], 0, 3, ("q", 0, 3))]
        for wt, dst, nm in ((wk, kT[1], "k"), (wq, qT[1], "q")):
            for nch in range(NCH):
                rest.append(proj_unit(wt, dst, 1, nch, (nm, 1, nch)))
        for i, u in enumerate(rest):
            p = 32 + 5 * i  # every 5th period from block 2 onward
            weave.setdefault((p // NT, p % NT), []).append(u)

        LA = 2  # scores lookahead depth
        nblocks = len(blocks)
        for j in range(LA):
            sp_ahead[(0, j)] = emit_scores(0, j)
        pending = []
        for b in range(nblocks):
            for jb in range(NT):
                emit_exp(b, jb, fetch_scores(b, jb))
                la = jb + LA
                if la < NT:
                    if (b, la) not in sp_ahead:
                        sp_ahead[(b, la)] = emit_scores(b, la)
                elif b + 1 < nblocks:
                    sp_ahead[(b + 1, la - NT)] = emit_scores(b + 1, la - NT)
                if jb == NT - 1 and b + 1 < nblocks:
                    # boundary prefetch into the idle third spool slot: keeps
                    # the exp stream covered across the PV bursts
                    sp_ahead[(b + 1, LA)] = emit_scores(b + 1, LA)
                # woven PE filler (deadlock-gated; up to 2 units per period)
                pending.extend(weave.pop((b, jb), ()))
                emitted = 0
                while pending and emitted < 2 and weave_safe((b, jb)):
                    pending.pop(0)((b, jb))
                    emitted += 1
                # PV bursts: q0-q2 inside the block, q3 deferred into the next
                # block (reads 4-period-old tiles, so no exp-latency exposure);
                # the final block's q3 is split so the tail is 2 matmuls long.
                if jb == 0 and b > 0:
                    emit_pv(b - 1, range(8, 12))
                elif jb == 2 and b > 0:
                    emit_pv(b - 1, range(12, NT), last=True)
                if jb in (8, 12):
                    emit_pv(b, range(jb - 8, jb - 4))
                if b == nblocks - 1:
                    if jb == 13:
                        emit_pv(b, range(8, 12))
                    elif jb == 14:
                        emit_pv(b, (12, 13))
                    elif jb == NT - 1:
                        emit_pv(b, (14, 15), last=True)
        assert not pending and not weave, (len(pending), sorted(weave))

    nc.compile()
    return nc


def _get_nc():
    if "nc" not in _CACHE:
        _CACHE["nc"] = _build_nc()
    return _CACHE["nc"]


def _prepare_in_maps(x, w_qkv):
    bf = ml_dtypes.bfloat16
    x = np.asarray(x, dtype=np.float32)
    w = np.asarray(w_qkv, dtype=np.float32)
    scale = DH ** -0.5
    in_maps = []
    # xT n-chunk-major: element (p, nch, kt, n) = x[b].T[kt*128+p, nch*512+n]
    xT_b = [
        np.ascontiguousarray(
            x[b].T.reshape(KT, P, NCH, 512).transpose(1, 2, 0, 3).reshape(P, KT * N)
        ).astype(bf)
        for b in range(B)
    ]
    for c in range(8):
        b, hg = divmod(c, 4)
        cs = slice(hg * HL * DH, (hg + 1) * HL * DH)
        in_maps.append(
            {
                "xT": xT_b[b],
                "wq": np.ascontiguousarray(w[:, cs] * scale).astype(bf),
                "wk": np.ascontiguousarray(w[:, 1024:2048][:, cs]).astype(bf),
                "wv": np.ascontiguousarray(w[:, 2048:3072][:, cs]).astype(bf),
            }
        )
    return in_maps


def _assemble(outs):
    full = np.empty((B, N, HEADS * DH), dtype=np.float32)
    for c in range(8):
        b, hg = divmod(c, 4)
        o = outs[c].reshape(HL, DH + 1, N)
        norm = o[:, :DH, :] / o[:, DH : DH + 1, :]  # [hl, d, n]
        full[b, :, hg * HL * DH : (hg + 1) * HL * DH] = norm.transpose(2, 0, 1).reshape(
            N, HL * DH
        )
    return full


def kernel(x, w_qkv):
    global LAST_RESULTS
    from concourse.bass_utils import run_bass_kernel_spmd

    nc = _get_nc()
    in_maps = _prepare_in_maps(x, w_qkv)
    last_err = None
    for _ in range(3):  # the runtime occasionally throws a transient device error
        try:
            res = run_bass_kernel_spmd(
                nc,
                in_maps,
                core_ids=list(range(8)),
                trace=TRACE,
                trace_cores=[0] if TRACE else None,
            )
            break
        except Exception as e:
            last_err = e
    else:
        raise last_err
    LAST_RESULTS = res
    return _assemble([r["out"] for r in res.results])
